# revision 3
# baseline (speedup 1.0000x reference)
"""ChamferLoss kernel for 8 Trainium2 NeuronCores.

Problem: pred (4,8192,3) f32, gt (4,8192,3) f32 ->
  loss = mean_b[ mean(pred2gt_b) + mean(gt2pred_b) + max(pred2gt_b) ]   (scalar f32)
where pred2gt[b,i] = min_j ||pred[b,i]-gt[b,j]||^2 and gt2pred[b,j] = min_i (same).

Sharding: data-parallel over B (2 cores per batch) x sequence-sharded rows.
Each core computes row-mins of two 4096x8192 distance blocks (dual orientation:
pred-half vs gt-full, and gt-half vs pred-full), so ALL reductions are free-axis
row reductions; the host does only the tiny final mean/max combines.

Distance computation: one K=18 bf16 matmul per tile via the augmented split-
precision form  d = nx + ny - 2 x.y  with x = xh + xl (bf16 hi/lo split) and
norms split into 3 bf16 parts; PSUM accumulates in fp32, so results are
fp32-accurate (abs err ~1e-4, dominated by the bf16-split representation).

Reduction: per 2048-column PSUM unit, ScalarE copies the upper 1024 columns to
SBUF; a custom fused DVE op (min body + min accumulate) then reduces the lower
1024 PSUM columns against the SBUF copy in a single 1x pass, draining PSUM
through both the DVE and ACT read ports concurrently.
"""

from contextlib import ExitStack

import numpy as np
import ml_dtypes

import concourse.bass as bass
import concourse.tile as tile
from concourse import bacc, mybir
from concourse import dve_ops
from concourse.bass_utils import run_bass_kernel_spmd
from concourse.dve_ops import DveOp
from concourse.dve_spec import Spec, Src0, Src1, C0, minn, lower
from concourse.dve_uop import DveOpSpec

B = 4
N = 8192          # pred points per batch
M = 8192          # gt points per batch
NCORES = 8
HALF = 4096       # rows per core per orientation
K = 18            # augmented contraction rows
ITILE = 128       # rows per matmul tile
NSTRIP = 512      # matmul moving free dim
UNIT = 2048       # columns per psum unit (4 matmuls, 4 banks)
NUNITS = M // UNIT          # 4 units per i-tile
NITILES = HALF // ITILE     # 32 i-tiles per orientation
BIG = 3.0e38

_bf16 = ml_dtypes.bfloat16


# --------------------------------------------------------------------------- #
# Custom fused DVE op: out = min(in0, in1); accum_out = min(s0, min_k out)
# --------------------------------------------------------------------------- #

def _ttmin_ref(in0, in1, s0, s1, imm2):
    out = np.minimum(in0.astype(np.float32), in1.astype(np.float32))
    s0v = s0 if np.ndim(s0) == 0 else np.asarray(s0).reshape(-1)
    return out, np.minimum(out.min(axis=-1), s0v)


def _register_min_op() -> DveOp:
    name = "TT_MIN_RED_ANT"
    for o in dve_ops.OPS:
        if o.name == name:
            return o
    spec = Spec(body=minn(Src0, Src1), accum=minn, accum_init=C0, reference=_ttmin_ref)
    shas = {}
    for ver in ("v3", "v4"):
        try:
            s = DveOpSpec(name=name, opcode=0, uops=lower(spec, ver=ver), rd1_en=True)
            shas[ver] = s.sha(ver)
        except Exception:
            pass
    op = DveOp(name, spec, subdim=False, uops_sha=shas)
    dve_ops.OPS.append(op)
    dve_ops._SUB_OPCODE_FOR_NAME[name] = dve_ops._CUSTOM_DVE_ROW_BASE + len(dve_ops.OPS) - 1
    dve_ops.CUSTOM_DVE_SPECS[name] = spec
    return op


# --------------------------------------------------------------------------- #
# Bass program (identical SPMD program on all 8 cores)
# --------------------------------------------------------------------------- #

_CACHE: dict = {}


def _build_program():
    op = _register_min_op()
    nc = bacc.Bacc("TRN2", target_bir_lowering=False, debug=False, num_devices=NCORES)

    ins = {}
    outs = {}
    for o in ("E", "F"):
        ins[f"lhsT_{o}"] = nc.dram_tensor(
            f"lhsT_{o}", [K, HALF], mybir.dt.bfloat16, kind="ExternalInput").ap()
        ins[f"rhs_{o}"] = nc.dram_tensor(
            f"rhs_{o}", [K, M], mybir.dt.bfloat16, kind="ExternalInput").ap()
        outs[o] = nc.dram_tensor(
            f"out{o}", [ITILE, NITILES], mybir.dt.float32, kind="ExternalOutput").ap()

    with tile.TileContext(nc) as tc:
        with ExitStack() as ctx:
            inp = ctx.enter_context(tc.tile_pool(name="inp", bufs=2))
            psum = ctx.enter_context(tc.tile_pool(name="psum", bufs=2, space="PSUM"))
            acp = ctx.enter_context(tc.tile_pool(name="acp", bufs=3))
            scr = ctx.enter_context(tc.tile_pool(name="scr", bufs=3))
            stp = ctx.enter_context(tc.tile_pool(name="stp", bufs=3))
            ost = ctx.enter_context(tc.tile_pool(name="ost", bufs=1))

            for o in ("E", "F"):
                lhsT = inp.tile([K, HALF], mybir.dt.bfloat16, tag="lhsT")
                nc.sync.dma_start(out=lhsT[:], in_=ins[f"lhsT_{o}"][:])
                rhs = inp.tile([K, M], mybir.dt.bfloat16, tag="rhs")
                nc.sync.dma_start(out=rhs[:], in_=ins[f"rhs_{o}"][:])

                outstage = ost.tile([ITILE, NITILES], mybir.dt.float32, tag="outstage")
                for t in range(NITILES):
                    w = lhsT[:, t * ITILE:(t + 1) * ITILE]
                    strip = stp.tile([ITILE, NUNITS], mybir.dt.float32, tag="strip")
                    for u in range(NUNITS):
                        pt = psum.tile([ITILE, UNIT], mybir.dt.float32, tag="pt")
                        for k in range(UNIT // NSTRIP):
                            j0 = u * UNIT + k * NSTRIP
                            nc.tensor.matmul(
                                pt[:, k * NSTRIP:(k + 1) * NSTRIP],
                                w, rhs[:, j0:j0 + NSTRIP],
                                start=True, stop=True)
                        cp = acp.tile([ITILE, UNIT // 2], mybir.dt.float32, tag="cp")
                        nc.scalar.copy(cp[:], pt[:, UNIT // 2:UNIT])
                        sc = scr.tile([ITILE, UNIT // 2], mybir.dt.bfloat16, tag="sc")
                        nc.vector._custom_dve(
                            op, out=sc[:], in0=pt[:, 0:UNIT // 2], in1=cp[:],
                            s0=BIG, accum_out=strip[:, u:u + 1])
                    nc.vector.tensor_reduce(
                        outstage[:, t:t + 1], strip[:],
                        axis=mybir.AxisListType.X, op=mybir.AluOpType.min)
                nc.sync.dma_start(out=outs[o][:], in_=outstage[:])

    nc.compile()
    return nc


# --------------------------------------------------------------------------- #
# Host-side input prep: augmented split-precision matrices
# --------------------------------------------------------------------------- #

def _split3(v):
    """Split fp32/fp64 array into 3 bf16 parts summing to ~v."""
    a = v.astype(_bf16).astype(np.float64)
    r = v - a
    b = r.astype(np.float32).astype(_bf16).astype(np.float64)
    c = (r - b).astype(np.float32).astype(_bf16).astype(np.float64)
    return a, b, c


def _augment(xrows, ycols):
    """Build (lhsT [K, nx], rhs [K, ny]) bf16 so that lhsT.T @ rhs [i,j]
    ~= ||x_i - y_j||^2 in fp32 precision.  xrows (nx,3), ycols (ny,3) f32."""
    nx_, ny_ = xrows.shape[0], ycols.shape[0]
    xh = xrows.astype(_bf16).astype(np.float64)
    xl32 = (xrows.astype(np.float64) - xh).astype(np.float32)
    xl = xl32.astype(_bf16).astype(np.float64)
    yh = ycols.astype(_bf16).astype(np.float64)
    yl32 = (ycols.astype(np.float64) - yh).astype(np.float32)
    yl = yl32.astype(_bf16).astype(np.float64)

    xe = xh + xl          # effective points (exactly representable as bf16+bf16)
    ye = yh + yl
    nxv = (xe * xe).sum(1)
    nyv = (ye * ye).sum(1)
    nxa, nxb, nxc = _split3(nxv)
    nya, nyb, nyc = _split3(nyv)

    lhsT = np.zeros((K, nx_), np.float32)
    rhs = np.zeros((K, ny_), np.float32)
    lhsT[0:3] = xh.T; rhs[0:3] = -2.0 * yh.T
    lhsT[3:6] = xh.T; rhs[3:6] = -2.0 * yl.T
    lhsT[6:9] = xl.T; rhs[6:9] = -2.0 * yh.T
    lhsT[9:12] = xl.T; rhs[9:12] = -2.0 * yl.T
    lhsT[12] = nxa; rhs[12] = 1.0
    lhsT[13] = nxb; rhs[13] = 1.0
    lhsT[14] = nxc; rhs[14] = 1.0
    lhsT[15] = 1.0; rhs[15] = nya
    lhsT[16] = 1.0; rhs[16] = nyb
    lhsT[17] = 1.0; rhs[17] = nyc
    return lhsT.astype(_bf16), rhs.astype(_bf16)


def _make_in_maps(pred, gt):
    in_maps = []
    rhs_gt = {}
    rhs_pred = {}
    for b in range(B):
        # rhs matrices are shared by the two cores of a batch; build once
        _, rhs_gt[b] = _augment(pred[b][:1], gt[b])
        _, rhs_pred[b] = _augment(gt[b][:1], pred[b])
    for c in range(NCORES):
        b, h = c // 2, c % 2
        rows = slice(h * HALF, (h + 1) * HALF)
        lhsT_E, _ = _augment(pred[b][rows], gt[b][:1])
        lhsT_F, _ = _augment(gt[b][rows], pred[b][:1])
        in_maps.append({
            "lhsT_E": lhsT_E, "rhs_E": rhs_gt[b],
            "lhsT_F": lhsT_F, "rhs_F": rhs_pred[b],
        })
    return in_maps


def _unstage(arr):
    """[128, 32] staging -> [4096] vector with row index t*128+p."""
    return np.asarray(arr, np.float32).T.reshape(-1)


def kernel(pred, gt):
    pred = np.asarray(pred, dtype=np.float32)
    gt = np.asarray(gt, dtype=np.float32)
    assert pred.shape == (B, N, 3) and gt.shape == (B, M, 3)

    if "nc" not in _CACHE:
        _CACHE["nc"] = _build_program()
    nc = _CACHE["nc"]

    in_maps = _make_in_maps(pred, gt)
    res = run_bass_kernel_spmd(nc, in_maps, list(range(NCORES)))

    loss_terms = []
    for b in range(B):
        p2g = np.concatenate([_unstage(res.results[2 * b]["outE"]),
                              _unstage(res.results[2 * b + 1]["outE"])])
        g2p = np.concatenate([_unstage(res.results[2 * b]["outF"]),
                              _unstage(res.results[2 * b + 1]["outF"])])
        loss_terms.append(p2g.mean(dtype=np.float64)
                          + g2p.mean(dtype=np.float64)
                          + np.float64(p2g.max()))
    return np.float32(np.mean(loss_terms))
